# revision 21
# baseline (speedup 1.0000x reference)
"""Trainium2 Bass kernel for nn_Llama_head (paired two-tower MLP head).

Computes sigmoid(rowwise_dot(mlp_u(xu), mlp_i(xv))) for N=32768 rows,
data-parallel across 8 NeuronCores (N sharded, weights replicated).

Key structure:
  - Host pre-packs x transposed/tiled so the kernel has no on-chip
    transposes: bf16 for k-tiles [0,26), fp8-e4m3 for k-tiles [26,32).
    The fp8 k-tiles run as DoubleRow (double-pumped) matmuls; w1 is
    pre-scaled by 512 so its values sit in e4m3's normal range, with
    the inverse folded into b1 (x512, relu is positively homogeneous)
    and w2 (/512). Measured end-to-end error ~1.3e-2 vs the 2e-2 gate.
  - Layer 1: hT[h, n] += w1[dk, h].T @ xT[dk, n] into PSUM (26 bf16
    k-tiles + 3 DoubleRow fp8 pairs); ACT relu (+512*b1) -> bf16 h;
    layer 2 + row-dot matmuls are emitted a few L1 matmuls into the
    next tower's PE stream so the PE never waits on ACT/DVE.
  - DMA: sync HWDGE ring starts ~6us earlier than the others, so it
    carries the need-ordered startup (w1u/xu0/w1i/xv0, k-chunked) and
    even blocks; the scalar ring carries odd blocks; the gpsimd SWDGE
    ring carries tiny constants and per-block 2KB outputs.
"""

import os

import numpy as np
import ml_dtypes

# Problem shape (hardcoded per harness contract).
N_FULL = 32768
D = 4096
H = 256
O = 64
N_CORES = 8

NC_ROWS = N_FULL // N_CORES  # rows per core
NB = 512                     # rows per block
NBLK = NC_ROWS // NB
KT = D // 128                # layer-1 k-tiles
KT8 = 12                     # k-tiles computed in fp8 (DoubleRow pairs)
KTB = KT - KT8               # k-tiles computed in bf16
HH_T = H // 128              # layer-2 k-tiles (= layer-1 out tiles)
W1SCALE = 512.0              # w1 pre-scale so fp8 tiles avoid subnormals
TRACE = bool(int(os.environ.get("KERNEL_TRACE", "0")))

LAST_RESULTS = None  # BassKernelResults of the most recent run (for profiling)

_PROGRAM = None


def _build_program():
    from contextlib import ExitStack

    import concourse.mybir as mybir
    import concourse.tile as tile
    from concourse import bacc

    f32 = mybir.dt.float32
    bf16 = mybir.dt.bfloat16
    fp8 = mybir.dt.float8e4
    AF = mybir.ActivationFunctionType
    DR = mybir.MatmulPerfMode.DoubleRow

    nc = bacc.Bacc("TRN2")

    xb_d = nc.dram_tensor("xb", [NBLK, 128, 2, KTB, NB], bf16, kind="ExternalInput")
    x8_d = nc.dram_tensor("x8", [NBLK, 128, 2, KT8, NB], fp8, kind="ExternalInput")
    TI = {"u": 0, "i": 1}
    w1b_d = {
        "u": nc.dram_tensor("w1ub", [128, HH_T, KTB, 128], bf16, kind="ExternalInput"),
        "i": nc.dram_tensor("w1ib", [128, HH_T, KTB, 128], bf16, kind="ExternalInput"),
    }
    w18_d = {
        "u": nc.dram_tensor("w1u8", [128, HH_T, KT8, 128], fp8, kind="ExternalInput"),
        "i": nc.dram_tensor("w1i8", [128, HH_T, KT8, 128], fp8, kind="ExternalInput"),
    }
    w2_d = {
        "u": nc.dram_tensor("w2u", [128, HH_T, O], bf16, kind="ExternalInput"),
        "i": nc.dram_tensor("w2i", [128, HH_T, O], bf16, kind="ExternalInput"),
    }
    cst_d = nc.dram_tensor("cst", [128, 6], f32, kind="ExternalInput")
    ones_d = nc.dram_tensor("ones", [O, 1], bf16, kind="ExternalInput")
    out = nc.dram_tensor("out", [NC_ROWS], f32, kind="ExternalOutput")

    with ExitStack() as ctx:
        tc = ctx.enter_context(tile.TileContext(nc))

        wpool = ctx.enter_context(tc.tile_pool(name="weights", bufs=1))
        xp = ctx.enter_context(tc.tile_pool(name="x", bufs=2))
        xp8 = ctx.enter_context(tc.tile_pool(name="x8", bufs=2))
        hp = ctx.enter_context(tc.tile_pool(name="h", bufs=4))
        uvp = ctx.enter_context(tc.tile_pool(name="uv", bufs=4))
        sp = ctx.enter_context(tc.tile_pool(name="sblk", bufs=2))
        ps_h = ctx.enter_context(tc.tile_pool(name="psh", bufs=4, space="PSUM"))
        ps_uv = ctx.enter_context(tc.tile_pool(name="psuv", bufs=2, space="PSUM"))
        ps_d = ctx.enter_context(tc.tile_pool(name="psd", bufs=2, space="PSUM"))

        w1b_sb = {}
        w18_sb = {}
        w2_sb = {}
        for s in ("u", "i"):
            w1b_sb[s] = wpool.tile([128, HH_T, KTB, 128], bf16, tag=f"w1b{s}", name=f"w1b{s}")
            w18_sb[s] = wpool.tile([128, HH_T, KT8, 128], fp8, tag=f"w18{s}", name=f"w18{s}")
            w2_sb[s] = wpool.tile([128, HH_T, O], bf16, tag=f"w2{s}", name=f"w2{s}")
        cst = wpool.tile([128, 6], f32, tag="cst", name="cst")
        ones_sb = wpool.tile([O, 1], bf16, tag="ones", name="ones")

        # Tiny constants + all fp8 x tiles + per-block outs ride the
        # gpsimd SWDGE ring (~30GB/s needed; the ring is otherwise idle).
        nc.gpsimd.dma_start(cst, cst_d[:])
        nc.gpsimd.dma_start(ones_sb, ones_d[:])

        b1_sb = {"u": cst[:, 0:2], "i": cst[:, 2:4]}
        b2_sb = {"u": cst[:O, 4:5], "i": cst[:O, 5:6]}

        nat_blk0 = xp.tile([128, 2, KTB, NB], bf16, tag="x", name="x")
        nat8_blk0 = xp8.tile([128, 2, KT8, NB], fp8, tag="x8", name="x8")

        # Startup: block-0 data striped across BOTH HWDGE rings in
        # k-chunks matching consumption order — each ring's queue is a
        # subsequence of the consumption sequence, so per-ring FIFO
        # completion order stays need-ordered and stalls stay short
        # (under the ~3.4us HAM re-throttle window). fp8 on gpsimd.
        nc.gpsimd.dma_start(w18_sb["u"], w18_d["u"][:])
        nc.gpsimd.dma_start(w18_sb["i"], w18_d["i"][:])
        nc.gpsimd.dma_start(nat8_blk0[:, 0], x8_d[0][:, 0])
        nc.gpsimd.dma_start(w2_sb["u"], w2_d["u"][:])
        nc.gpsimd.dma_start(w2_sb["i"], w2_d["i"][:])
        nc.gpsimd.dma_start(nat8_blk0[:, 1], x8_d[0][:, 1])

        nc.sync.dma_start(w1b_sb["u"][:, 0, 0:4, :], w1b_d["u"][:, 0, 0:4, :])
        nc.sync.dma_start(nat_blk0[:, 0, 0:4, :], xb_d[0][:, 0, 0:4, :])
        nc.scalar.dma_start(nat_blk0[:, 0, 4:8, :], xb_d[0][:, 0, 4:8, :])
        nc.sync.dma_start(w1b_sb["u"][:, 0, 4:KTB, :], w1b_d["u"][:, 0, 4:KTB, :])
        nc.scalar.dma_start(w1b_sb["u"][:, 1], w1b_d["u"][:, 1])
        nc.sync.dma_start(nat_blk0[:, 0, 8:16, :], xb_d[0][:, 0, 8:16, :])
        nc.scalar.dma_start(nat_blk0[:, 0, 16:KTB, :], xb_d[0][:, 0, 16:KTB, :])
        nc.sync.dma_start(w1b_sb["i"][:, 0], w1b_d["i"][:, 0])
        nc.sync.dma_start(nat_blk0[:, 1, 0:8, :], xb_d[0][:, 1, 0:8, :])
        nc.scalar.dma_start(nat_blk0[:, 1, 8:16, :], xb_d[0][:, 1, 8:16, :])
        nc.scalar.dma_start(w1b_sb["i"][:, 1], w1b_d["i"][:, 1])
        nc.sync.dma_start(nat_blk0[:, 1, 16:KTB, :], xb_d[0][:, 1, 16:KTB, :])

        # --- deferred PE emission machinery (see module docstring).
        pending = []

        def after_mms(n, fn):
            pending.append([n, fn])

        def tick():
            due = [it for it in pending if it[0] <= 1]
            for it in due:
                pending.remove(it)
                it[1]()
            for it in pending:
                it[0] -= 1

        def flush():
            while pending:
                pending.pop(0)[1]()

        for b in range(NBLK):
            stash = {}
            if b == 0:
                natp = nat_blk0
                nat8p = nat8_blk0
            else:
                natp = xp.tile([128, 2, KTB, NB], bf16, tag="x", name="x")
                if b == 1:
                    # Block 1 tower-halves striped across both rings.
                    nc.sync.dma_start(natp[:, 0, 0:12, :], xb_d[b][:, 0, 0:12, :])
                    nc.scalar.dma_start(natp[:, 0, 12:, :], xb_d[b][:, 0, 12:, :])
                    nc.sync.dma_start(natp[:, 1, 0:12, :], xb_d[b][:, 1, 0:12, :])
                    nc.scalar.dma_start(natp[:, 1, 12:, :], xb_d[b][:, 1, 12:, :])
                else:
                    # One whole-block DMA (both towers), alternating rings.
                    eng = nc.sync if b % 2 == 0 else nc.scalar
                    eng.dma_start(natp, xb_d[b])
                nat8p = xp8.tile([128, 2, KT8, NB], fp8, tag="x8", name="x8")
                nc.gpsimd.dma_start(nat8p, x8_d[b])
            hsb_all = {}
            for sname in ("u", "i"):
                nat = natp[:, TI[sname]]
                nat8 = nat8p[:, TI[sname]]

                ph = [ps_h.tile([128, NB], f32, tag="ph", name="ph") for _ in range(HH_T)]
                hsb = [hp.tile([128, NB], bf16, tag="h", name="h") for _ in range(HH_T)]
                for hh in range(HH_T):
                    for k in range(KTB):
                        nc.tensor.matmul(
                            ph[hh],
                            w1b_sb[sname][:, hh, k, :],
                            nat[:, k, :],
                            start=(k == 0),
                            stop=False,
                        )
                        tick()
                    for j in range(KT8 // 2):
                        nc.tensor.matmul(
                            ph[hh],
                            w18_sb[sname][:, hh, 2 * j : 2 * j + 2, :],
                            nat8[:, 2 * j : 2 * j + 2, :],
                            start=False,
                            stop=(j == KT8 // 2 - 1),
                            perf_mode=DR,
                        )
                        tick()
                    nc.scalar.activation(
                        hsb[hh], ph[hh], AF.Relu, bias=b1_sb[sname][:, hh : hh + 1]
                    )

                hsb_all[sname] = hsb

            # Both towers' layer-2 matmuls in one back-to-back burst so
            # only the first pays the isolated-matmul fill/drain penalty.
            def l2burst(hsb_all=hsb_all, stash=stash):
                puvs = {}
                for s in ("u", "i"):
                    puvs[s] = ps_uv.tile([O, NB], f32, tag="puv", name="puv")
                    for hh in range(HH_T):
                        nc.tensor.matmul(
                            puvs[s],
                            w2_sb[s][:, hh, :],
                            hsb_all[s][hh],
                            start=(hh == 0),
                            stop=(hh == HH_T - 1),
                        )
                for s in ("u", "i"):
                    usb = uvp.tile([O, NB], bf16, tag="uv", name="uv")
                    nc.vector.tensor_scalar_add(usb, puvs[s], b2_sb[s])
                    stash[s] = usb

            after_mms(3, l2burst)

            def dot(b=b, stash=stash):
                prod = uvp.tile([O, NB], bf16, tag="prod", name="prod")
                nc.vector.tensor_mul(prod, stash["u"], stash["i"])
                pd = ps_d.tile([1, NB], f32, tag="pd", name="pd")
                nc.tensor.matmul(pd, ones_sb, prod, start=True, stop=True)
                s_blk = sp.tile([1, NB], f32, tag="sblk", name="s_blk")
                nc.scalar.activation(s_blk, pd, AF.Sigmoid)
                # Final block's output goes out on the (long-idle) sync
                # ring for the lightest completion path at kernel end.
                eng = nc.sync if b == NBLK - 1 else nc.gpsimd
                eng.dma_start(out[b * NB : (b + 1) * NB], s_blk)

            after_mms(10, dot)

        flush()

    nc.compile()
    return nc


def _pack_x(xu, xv):
    """Both towers -> per-core ([NBLK,128,2,KTB,NB] bf16, [NBLK,128,2,KT8,NB] fp8).

    packed_b[c][b, p, t, k, n] = x_t[c*NC_ROWS + b*NB + n, k*128 + p]   (k < KTB)
    packed_8[c][b, p, t, j, n] = x_t[c*NC_ROWS + b*NB + n, (KTB+j)*128 + p]
    """
    outs_b, outs_8 = [], []
    parts = []
    for x in (xu, xv):
        xf = np.asarray(x, dtype=np.float32)
        xb = xf[:, : KTB * 128].astype(ml_dtypes.bfloat16)
        x8 = xf[:, KTB * 128 :].astype(ml_dtypes.float8_e4m3fn)
        parts.append(
            (
                xb.reshape(N_CORES, NBLK, NB, KTB, 128),
                x8.reshape(N_CORES, NBLK, NB, KT8, 128),
            )
        )
    for c in range(N_CORES):
        # [b, p, t, k, n] from per-tower [b, n, k, p]
        xb = np.stack(
            [parts[0][0][c].transpose(0, 3, 2, 1), parts[1][0][c].transpose(0, 3, 2, 1)],
            axis=2,
        )
        x8 = np.stack(
            [parts[0][1][c].transpose(0, 3, 2, 1), parts[1][1][c].transpose(0, 3, 2, 1)],
            axis=2,
        )
        outs_b.append(np.ascontiguousarray(xb))
        outs_8.append(np.ascontiguousarray(x8))
    return outs_b, outs_8


def _pack_w1(w1):
    """[D, H] -> hh-major ([128, HH_T, KTB, 128] bf16, [128, HH_T, KT8, 128] fp8).

    element (p, a, k, m) = W1SCALE * w1[k*128 + p, a*128 + m]
    """
    wf = np.asarray(w1, dtype=np.float32) * np.float32(W1SCALE)
    wb = wf[: KTB * 128].astype(ml_dtypes.bfloat16)
    w8 = wf[KTB * 128 :].astype(ml_dtypes.float8_e4m3fn)
    wb = wb.reshape(KTB, 128, HH_T, 128).transpose(1, 2, 0, 3)
    w8 = w8.reshape(KT8, 128, HH_T, 128).transpose(1, 2, 0, 3)
    return np.ascontiguousarray(wb), np.ascontiguousarray(w8)


def _pack_w2(w2):
    """[H, O] -> [128, HH_T, O] bf16, scaled by 1/W1SCALE."""
    wb = (np.asarray(w2, dtype=np.float32) / np.float32(W1SCALE)).astype(
        ml_dtypes.bfloat16
    )
    return np.ascontiguousarray(wb.reshape(HH_T, 128, O).transpose(1, 0, 2))


def _pack_cst(b1u, b1i, b2u, b2i):
    """[128, 6] f32: W1SCALE*b1u (2 cols), W1SCALE*b1i (2 cols), b2u, b2i."""
    cst = np.zeros((128, 6), dtype=np.float32)
    cst[:, 0:2] = np.float32(W1SCALE) * b1u.reshape(2, 128).T
    cst[:, 2:4] = np.float32(W1SCALE) * b1i.reshape(2, 128).T
    cst[: b2u.shape[0], 4] = b2u
    cst[: b2i.shape[0], 5] = b2i
    return cst


def _get_program():
    global _PROGRAM
    if _PROGRAM is None:
        _PROGRAM = _build_program()
    return _PROGRAM


def kernel(
    user_origin_emb,
    item_origin_emb,
    u_w1,
    u_b1,
    u_w2,
    u_b2,
    i_w1,
    i_b1,
    i_w2,
    i_b2,
):
    global LAST_RESULTS
    from concourse.bass_utils import run_bass_kernel_spmd

    xb_packed, x8_packed = _pack_x(user_origin_emb, item_origin_emb)
    w1ub, w1u8 = _pack_w1(u_w1)
    w1ib, w1i8 = _pack_w1(i_w1)
    shared = {
        "w1ub": w1ub,
        "w1u8": w1u8,
        "w1ib": w1ib,
        "w1i8": w1i8,
        "w2u": _pack_w2(u_w2),
        "w2i": _pack_w2(i_w2),
        "cst": _pack_cst(
            np.asarray(u_b1, dtype=np.float32),
            np.asarray(i_b1, dtype=np.float32),
            np.asarray(u_b2, dtype=np.float32),
            np.asarray(i_b2, dtype=np.float32),
        ),
        "ones": np.ones((O, 1), dtype=ml_dtypes.bfloat16),
    }

    nc = _get_program()
    in_maps = [
        {"xb": xb_packed[c], "x8": x8_packed[c], **shared}
        for c in range(N_CORES)
    ]
    res = run_bass_kernel_spmd(nc, in_maps, core_ids=list(range(N_CORES)), trace=TRACE)
    LAST_RESULTS = res
    return np.concatenate([r["out"] for r in res.results], axis=0)


# revision 22
# speedup vs baseline: 1.1346x; 1.1346x over previous
"""Trainium2 Bass kernel for nn_Llama_head (paired two-tower MLP head).

Computes sigmoid(rowwise_dot(mlp_u(xu), mlp_i(xv))) for N=32768 rows,
data-parallel across 8 NeuronCores (N sharded, weights replicated).

Key structure:
  - Host pre-packs x transposed/tiled so the kernel has no on-chip
    transposes: bf16 for k-tiles [0,26), fp8-e4m3 for k-tiles [26,32).
    The fp8 k-tiles run as DoubleRow (double-pumped) matmuls; w1 is
    pre-scaled by 512 so its values sit in e4m3's normal range, with
    the inverse folded into b1 (x512, relu is positively homogeneous)
    and w2 (/512). Measured end-to-end error ~1.3e-2 vs the 2e-2 gate.
  - Layer 1: hT[h, n] += w1[dk, h].T @ xT[dk, n] into PSUM (26 bf16
    k-tiles + 3 DoubleRow fp8 pairs); ACT relu (+512*b1) -> bf16 h;
    layer 2 + row-dot matmuls are emitted a few L1 matmuls into the
    next tower's PE stream so the PE never waits on ACT/DVE.
  - DMA: sync HWDGE ring starts ~6us earlier than the others, so it
    carries the need-ordered startup (w1u/xu0/w1i/xv0, k-chunked) and
    even blocks; the scalar ring carries odd blocks; the gpsimd SWDGE
    ring carries tiny constants and per-block 2KB outputs.
"""

import os

import numpy as np
import ml_dtypes

# Problem shape (hardcoded per harness contract).
N_FULL = 32768
D = 4096
H = 256
O = 64
N_CORES = 8

NC_ROWS = N_FULL // N_CORES  # rows per core
NB = 512                     # rows per block
NBLK = NC_ROWS // NB
KT = D // 128                # layer-1 k-tiles
KT8 = 10                     # k-tiles computed in fp8 (DoubleRow pairs)
KTB = KT - KT8               # k-tiles computed in bf16
HH_T = H // 128              # layer-2 k-tiles (= layer-1 out tiles)
W1SCALE = 512.0              # w1 pre-scale so fp8 tiles avoid subnormals
TRACE = bool(int(os.environ.get("KERNEL_TRACE", "0")))

LAST_RESULTS = None  # BassKernelResults of the most recent run (for profiling)

_PROGRAM = None


def _build_program():
    from contextlib import ExitStack

    import concourse.mybir as mybir
    import concourse.tile as tile
    from concourse import bacc

    f32 = mybir.dt.float32
    bf16 = mybir.dt.bfloat16
    fp8 = mybir.dt.float8e4
    AF = mybir.ActivationFunctionType
    DR = mybir.MatmulPerfMode.DoubleRow

    nc = bacc.Bacc("TRN2")

    xb_d = nc.dram_tensor("xb", [NBLK, 128, 2, KTB, NB], bf16, kind="ExternalInput")
    x8_d = nc.dram_tensor("x8", [NBLK, 128, 2, KT8, NB], fp8, kind="ExternalInput")
    TI = {"u": 0, "i": 1}
    w1b_d = {
        "u": nc.dram_tensor("w1ub", [128, HH_T, KTB, 128], bf16, kind="ExternalInput"),
        "i": nc.dram_tensor("w1ib", [128, HH_T, KTB, 128], bf16, kind="ExternalInput"),
    }
    w18_d = {
        "u": nc.dram_tensor("w1u8", [128, HH_T, KT8, 128], fp8, kind="ExternalInput"),
        "i": nc.dram_tensor("w1i8", [128, HH_T, KT8, 128], fp8, kind="ExternalInput"),
    }
    w2_d = {
        "u": nc.dram_tensor("w2u", [128, HH_T, O], bf16, kind="ExternalInput"),
        "i": nc.dram_tensor("w2i", [128, HH_T, O], bf16, kind="ExternalInput"),
    }
    cst_d = nc.dram_tensor("cst", [128, 6], f32, kind="ExternalInput")
    ones_d = nc.dram_tensor("ones", [O, 1], bf16, kind="ExternalInput")
    out = nc.dram_tensor("out", [NC_ROWS], f32, kind="ExternalOutput")

    with ExitStack() as ctx:
        tc = ctx.enter_context(tile.TileContext(nc))

        wpool = ctx.enter_context(tc.tile_pool(name="weights", bufs=1))
        xp = ctx.enter_context(tc.tile_pool(name="x", bufs=2))
        xp8 = ctx.enter_context(tc.tile_pool(name="x8", bufs=2))
        hp = ctx.enter_context(tc.tile_pool(name="h", bufs=4))
        uvp = ctx.enter_context(tc.tile_pool(name="uv", bufs=4))
        sp = ctx.enter_context(tc.tile_pool(name="sblk", bufs=2))
        ps_h = ctx.enter_context(tc.tile_pool(name="psh", bufs=4, space="PSUM"))
        ps_uv = ctx.enter_context(tc.tile_pool(name="psuv", bufs=2, space="PSUM"))
        ps_d = ctx.enter_context(tc.tile_pool(name="psd", bufs=2, space="PSUM"))

        w1b_sb = {}
        w18_sb = {}
        w2_sb = {}
        for s in ("u", "i"):
            w1b_sb[s] = wpool.tile([128, HH_T, KTB, 128], bf16, tag=f"w1b{s}", name=f"w1b{s}")
            w18_sb[s] = wpool.tile([128, HH_T, KT8, 128], fp8, tag=f"w18{s}", name=f"w18{s}")
            w2_sb[s] = wpool.tile([128, HH_T, O], bf16, tag=f"w2{s}", name=f"w2{s}")
        cst = wpool.tile([128, 6], f32, tag="cst", name="cst")
        ones_sb = wpool.tile([O, 1], bf16, tag="ones", name="ones")

        # Tiny constants + all fp8 x tiles + per-block outs ride the
        # gpsimd SWDGE ring (~30GB/s needed; the ring is otherwise idle).
        nc.gpsimd.dma_start(cst, cst_d[:])
        nc.gpsimd.dma_start(ones_sb, ones_d[:])

        b1_sb = {"u": cst[:, 0:2], "i": cst[:, 2:4]}
        b2_sb = {"u": cst[:O, 4:5], "i": cst[:O, 5:6]}

        nat_blk0 = xp.tile([128, 2, KTB, NB], bf16, tag="x", name="x")
        nat8_blk0 = xp8.tile([128, 2, KT8, NB], fp8, tag="x8", name="x8")

        # Startup: block-0 data striped across BOTH HWDGE rings in
        # k-chunks matching consumption order — each ring's queue is a
        # subsequence of the consumption sequence, so per-ring FIFO
        # completion order stays need-ordered and stalls stay short
        # (under the ~3.4us HAM re-throttle window). fp8 on gpsimd.
        nc.gpsimd.dma_start(w18_sb["u"], w18_d["u"][:])
        nc.gpsimd.dma_start(w18_sb["i"], w18_d["i"][:])
        nc.gpsimd.dma_start(nat8_blk0[:, 0], x8_d[0][:, 0])
        nc.gpsimd.dma_start(w2_sb["u"], w2_d["u"][:])
        nc.gpsimd.dma_start(w2_sb["i"], w2_d["i"][:])
        nc.gpsimd.dma_start(nat8_blk0[:, 1], x8_d[0][:, 1])

        nc.sync.dma_start(w1b_sb["u"][:, 0, 0:4, :], w1b_d["u"][:, 0, 0:4, :])
        nc.sync.dma_start(nat_blk0[:, 0, 0:4, :], xb_d[0][:, 0, 0:4, :])
        nc.scalar.dma_start(nat_blk0[:, 0, 4:8, :], xb_d[0][:, 0, 4:8, :])
        nc.sync.dma_start(w1b_sb["u"][:, 0, 4:KTB, :], w1b_d["u"][:, 0, 4:KTB, :])
        nc.scalar.dma_start(w1b_sb["u"][:, 1], w1b_d["u"][:, 1])
        nc.sync.dma_start(nat_blk0[:, 0, 8:16, :], xb_d[0][:, 0, 8:16, :])
        nc.scalar.dma_start(nat_blk0[:, 0, 16:KTB, :], xb_d[0][:, 0, 16:KTB, :])
        nc.sync.dma_start(w1b_sb["i"][:, 0], w1b_d["i"][:, 0])
        nc.sync.dma_start(nat_blk0[:, 1, 0:8, :], xb_d[0][:, 1, 0:8, :])
        nc.scalar.dma_start(nat_blk0[:, 1, 8:16, :], xb_d[0][:, 1, 8:16, :])
        nc.scalar.dma_start(w1b_sb["i"][:, 1], w1b_d["i"][:, 1])
        nc.sync.dma_start(nat_blk0[:, 1, 16:KTB, :], xb_d[0][:, 1, 16:KTB, :])

        # --- deferred PE emission machinery (see module docstring).
        pending = []

        def after_mms(n, fn):
            pending.append([n, fn])

        def tick():
            due = [it for it in pending if it[0] <= 1]
            for it in due:
                pending.remove(it)
                it[1]()
            for it in pending:
                it[0] -= 1

        def flush():
            while pending:
                pending.pop(0)[1]()

        for b in range(NBLK):
            stash = {}
            if b == 0:
                natp = nat_blk0
                nat8p = nat8_blk0
            else:
                natp = xp.tile([128, 2, KTB, NB], bf16, tag="x", name="x")
                if b == 1:
                    # Block 1 tower-halves striped across both rings.
                    nc.sync.dma_start(natp[:, 0, 0:12, :], xb_d[b][:, 0, 0:12, :])
                    nc.scalar.dma_start(natp[:, 0, 12:, :], xb_d[b][:, 0, 12:, :])
                    nc.sync.dma_start(natp[:, 1, 0:12, :], xb_d[b][:, 1, 0:12, :])
                    nc.scalar.dma_start(natp[:, 1, 12:, :], xb_d[b][:, 1, 12:, :])
                else:
                    # One whole-block DMA (both towers), alternating rings.
                    eng = nc.sync if b % 2 == 0 else nc.scalar
                    eng.dma_start(natp, xb_d[b])
                nat8p = xp8.tile([128, 2, KT8, NB], fp8, tag="x8", name="x8")
                nc.gpsimd.dma_start(nat8p, x8_d[b])
            hsb_all = {}
            for sname in ("u", "i"):
                nat = natp[:, TI[sname]]
                nat8 = nat8p[:, TI[sname]]

                ph = [ps_h.tile([128, NB], f32, tag="ph", name="ph") for _ in range(HH_T)]
                hsb = [hp.tile([128, NB], bf16, tag="h", name="h") for _ in range(HH_T)]
                for hh in range(HH_T):
                    for k in range(KTB):
                        nc.tensor.matmul(
                            ph[hh],
                            w1b_sb[sname][:, hh, k, :],
                            nat[:, k, :],
                            start=(k == 0),
                            stop=False,
                        )
                        tick()
                    for j in range(KT8 // 2):
                        nc.tensor.matmul(
                            ph[hh],
                            w18_sb[sname][:, hh, 2 * j : 2 * j + 2, :],
                            nat8[:, 2 * j : 2 * j + 2, :],
                            start=False,
                            stop=(j == KT8 // 2 - 1),
                            perf_mode=DR,
                        )
                        tick()
                    nc.scalar.activation(
                        hsb[hh], ph[hh], AF.Relu, bias=b1_sb[sname][:, hh : hh + 1]
                    )

                hsb_all[sname] = hsb

            # Both towers' layer-2 matmuls in one back-to-back burst so
            # only the first pays the isolated-matmul fill/drain penalty.
            def l2burst(hsb_all=hsb_all, stash=stash):
                puvs = {}
                for s in ("u", "i"):
                    puvs[s] = ps_uv.tile([O, NB], f32, tag="puv", name="puv")
                    for hh in range(HH_T):
                        nc.tensor.matmul(
                            puvs[s],
                            w2_sb[s][:, hh, :],
                            hsb_all[s][hh],
                            start=(hh == 0),
                            stop=(hh == HH_T - 1),
                        )
                for s in ("u", "i"):
                    usb = uvp.tile([O, NB], bf16, tag="uv", name="uv")
                    nc.vector.tensor_scalar_add(usb, puvs[s], b2_sb[s])
                    stash[s] = usb

            after_mms(3, l2burst)

            def dot(b=b, stash=stash):
                prod = uvp.tile([O, NB], bf16, tag="prod", name="prod")
                nc.vector.tensor_mul(prod, stash["u"], stash["i"])
                pd = ps_d.tile([1, NB], f32, tag="pd", name="pd")
                nc.tensor.matmul(pd, ones_sb, prod, start=True, stop=True)
                s_blk = sp.tile([1, NB], f32, tag="sblk", name="s_blk")
                nc.scalar.activation(s_blk, pd, AF.Sigmoid)
                # Final block's output goes out on the (long-idle) sync
                # ring for the lightest completion path at kernel end.
                eng = nc.sync if b == NBLK - 1 else nc.gpsimd
                eng.dma_start(out[b * NB : (b + 1) * NB], s_blk)

            after_mms(10, dot)

        flush()

    nc.compile()
    return nc


def _pack_x(xu, xv):
    """Both towers -> per-core ([NBLK,128,2,KTB,NB] bf16, [NBLK,128,2,KT8,NB] fp8).

    packed_b[c][b, p, t, k, n] = x_t[c*NC_ROWS + b*NB + n, k*128 + p]   (k < KTB)
    packed_8[c][b, p, t, j, n] = x_t[c*NC_ROWS + b*NB + n, (KTB+j)*128 + p]
    """
    outs_b, outs_8 = [], []
    parts = []
    for x in (xu, xv):
        xf = np.asarray(x, dtype=np.float32)
        xb = xf[:, : KTB * 128].astype(ml_dtypes.bfloat16)
        x8 = xf[:, KTB * 128 :].astype(ml_dtypes.float8_e4m3fn)
        parts.append(
            (
                xb.reshape(N_CORES, NBLK, NB, KTB, 128),
                x8.reshape(N_CORES, NBLK, NB, KT8, 128),
            )
        )
    for c in range(N_CORES):
        # [b, p, t, k, n] from per-tower [b, n, k, p]
        xb = np.stack(
            [parts[0][0][c].transpose(0, 3, 2, 1), parts[1][0][c].transpose(0, 3, 2, 1)],
            axis=2,
        )
        x8 = np.stack(
            [parts[0][1][c].transpose(0, 3, 2, 1), parts[1][1][c].transpose(0, 3, 2, 1)],
            axis=2,
        )
        outs_b.append(np.ascontiguousarray(xb))
        outs_8.append(np.ascontiguousarray(x8))
    return outs_b, outs_8


def _pack_w1(w1):
    """[D, H] -> hh-major ([128, HH_T, KTB, 128] bf16, [128, HH_T, KT8, 128] fp8).

    element (p, a, k, m) = W1SCALE * w1[k*128 + p, a*128 + m]
    """
    wf = np.asarray(w1, dtype=np.float32) * np.float32(W1SCALE)
    wb = wf[: KTB * 128].astype(ml_dtypes.bfloat16)
    w8 = wf[KTB * 128 :].astype(ml_dtypes.float8_e4m3fn)
    wb = wb.reshape(KTB, 128, HH_T, 128).transpose(1, 2, 0, 3)
    w8 = w8.reshape(KT8, 128, HH_T, 128).transpose(1, 2, 0, 3)
    return np.ascontiguousarray(wb), np.ascontiguousarray(w8)


def _pack_w2(w2):
    """[H, O] -> [128, HH_T, O] bf16, scaled by 1/W1SCALE."""
    wb = (np.asarray(w2, dtype=np.float32) / np.float32(W1SCALE)).astype(
        ml_dtypes.bfloat16
    )
    return np.ascontiguousarray(wb.reshape(HH_T, 128, O).transpose(1, 0, 2))


def _pack_cst(b1u, b1i, b2u, b2i):
    """[128, 6] f32: W1SCALE*b1u (2 cols), W1SCALE*b1i (2 cols), b2u, b2i."""
    cst = np.zeros((128, 6), dtype=np.float32)
    cst[:, 0:2] = np.float32(W1SCALE) * b1u.reshape(2, 128).T
    cst[:, 2:4] = np.float32(W1SCALE) * b1i.reshape(2, 128).T
    cst[: b2u.shape[0], 4] = b2u
    cst[: b2i.shape[0], 5] = b2i
    return cst


def _get_program():
    global _PROGRAM
    if _PROGRAM is None:
        _PROGRAM = _build_program()
    return _PROGRAM


def kernel(
    user_origin_emb,
    item_origin_emb,
    u_w1,
    u_b1,
    u_w2,
    u_b2,
    i_w1,
    i_b1,
    i_w2,
    i_b2,
):
    global LAST_RESULTS
    from concourse.bass_utils import run_bass_kernel_spmd

    xb_packed, x8_packed = _pack_x(user_origin_emb, item_origin_emb)
    w1ub, w1u8 = _pack_w1(u_w1)
    w1ib, w1i8 = _pack_w1(i_w1)
    shared = {
        "w1ub": w1ub,
        "w1u8": w1u8,
        "w1ib": w1ib,
        "w1i8": w1i8,
        "w2u": _pack_w2(u_w2),
        "w2i": _pack_w2(i_w2),
        "cst": _pack_cst(
            np.asarray(u_b1, dtype=np.float32),
            np.asarray(i_b1, dtype=np.float32),
            np.asarray(u_b2, dtype=np.float32),
            np.asarray(i_b2, dtype=np.float32),
        ),
        "ones": np.ones((O, 1), dtype=ml_dtypes.bfloat16),
    }

    nc = _get_program()
    in_maps = [
        {"xb": xb_packed[c], "x8": x8_packed[c], **shared}
        for c in range(N_CORES)
    ]
    res = run_bass_kernel_spmd(nc, in_maps, core_ids=list(range(N_CORES)), trace=TRACE)
    LAST_RESULTS = res
    return np.concatenate([r["out"] for r in res.results], axis=0)
